# revision 29
# baseline (speedup 1.0000x reference)
"""APIQMixer Trainium2 kernel — 8-core data-parallel over the b*t axis.

Per core (nbt=2048 rows, 4 free-dim tiles of 512), transposed layout
(features on partitions, bt on the free dim).  All matmul operands bf16;
PSUM + mixing fp32.  The schedule is built around PE streaming passes:
a "pass" is one rhs stream through the PE array; extra matmuls whose
weight tiles sit at disjoint (row, col) rectangles ride the same pass
for free.

  - L1 hypernet: per stack of 4 agents, 4 passes (mc chunks), each with
    4 row-tiled riders (tile_position=(32r,0)).  Stack 5 (1 agent) is
    DMA-replicated 4x on partitions so its 4 mc chunks ride ONE pass.
  - ReLU PSUM->SBUF drains alternate ACT/DVE; L1(s+1) passes are
    interleaved between L2(s) agent passes so the PE never waits.
  - Ally L2: per agent/kc, ONE h1 pass carries the per-agent w matmul
    (cols 64:128 -> half-bank pwa slot) + the b accumulation rider
    (cols 0:64 -> pwe).  |pwa|*qv products fold back into pwe[0:64]
    through K=64 PE passes (identity weights at partitions 64:128), so
    hid_a accumulates in PSUM and no vector-add chain exists.
  - Enemy L2: h1 pass = b_e rider (cols 0:64) + w_e rider (cols 64:128);
    h2 pass = pl2e-w rider + scalar-bias rider into pl2e[64].
  - All scalar biases (b2a, b2e) and the final dot accumulate into
    pl2e[64]; the output is one ACT op + DMA.
  - PSUM map (8 banks): ph x4 | pwe | bank56 x2 (pl2a-w / pwa slots /
    pq) | pl2e.
"""

import numpy as np

import concourse.bass as bass
from concourse import bacc
import concourse.mybir as mybir
from concourse.bass import ds, ts
from concourse.bass_utils import run_bass_kernel_spmd
from concourse.tile import TileContext

F32 = mybir.dt.float32
BF16 = mybir.dt.bfloat16
AF = mybir.ActivationFunctionType
OP = mybir.AluOpType

A, NE = 10, 11
FA = FE = 32
E, H = 64, 256
B, T = 128, 128
BT = B * T
SD = A * FA + NE * FE   # 672
NCORES = 8
NBT = BT // NCORES      # 2048 rows per core
NF = 512                # free-dim tile
NT = NBT // NF          # 4 tiles
NAG = A + NE            # 21 agents

W_NAMES = [
    "l1a_w1", "l1a_w2", "l1e_w1", "l1e_w2",
    "l2a_w1", "l2a_w2", "l2e_w1", "l2e_w2",
]


def _mm(nc, out, lhsT, rhs, **kw):
    nc.tensor.matmul(out, lhsT, rhs, **kw)


def build():
    nc = bacc.Bacc()

    qvT_e = nc.declare_dram_parameter("qvT", [A, NBT], BF16, isOutput=False)
    qvB_e = nc.declare_dram_parameter("qvB", [A, E, NBT], BF16, isOutput=False)
    stT_e = nc.declare_dram_parameter("statesT", [SD, NBT], BF16, isOutput=False)
    w1a4_e = nc.declare_dram_parameter("w1a4", [128, 2 * H], BF16, isOutput=False)
    w1e4_e = nc.declare_dram_parameter("w1e4", [128, 2 * H], BF16, isOutput=False)
    w2l1a_e = nc.declare_dram_parameter("w2l1a", [128, 256], BF16, isOutput=False)
    w2l1e_e = nc.declare_dram_parameter("w2l1e", [128, 256], BF16, isOutput=False)
    w2l2a_e = nc.declare_dram_parameter("w2l2a", [128, 256], BF16, isOutput=False)
    w2l2e_e = nc.declare_dram_parameter("w2l2e", [128, 256], BF16, isOutput=False)
    bvec_e = nc.declare_dram_parameter("bvec", [128, 14], F32, isOutput=False)
    cmat_e = nc.declare_dram_parameter("cmat", [128, 385], BF16, isOutput=False)
    out_e = nc.declare_dram_parameter("out", [NBT], F32, isOutput=True)

    from contextlib import ExitStack
    with TileContext(nc) as tc, ExitStack() as ctx:
        const = ctx.enter_context(tc.tile_pool(name="const", bufs=1))
        hpool = ctx.enter_context(tc.tile_pool(name="hpool", bufs=24))
        qbp = ctx.enter_context(tc.tile_pool(name="qbp", bufs=3))
        mix = ctx.enter_context(tc.tile_pool(name="mix", bufs=2))
        ph = ctx.enter_context(tc.tile_pool(name="ph", bufs=4, space="PSUM"))
        pacc = ctx.enter_context(tc.tile_pool(name="pacc", bufs=1, space="PSUM"))

        # ---------------- static loads (first-use order, no barrier) ----
        stacks = [None] * 6
        w1a4 = const.tile([128, 2 * H], BF16, name="w1a4")
        nc.sync.dma_start(out=w1a4[:, :], in_=w1a4_e[:, :])
        bvec = const.tile([128, 14], F32, name="bvec")
        nc.sync.dma_start(out=bvec[:, :], in_=bvec_e[:, :])
        for s in (0, 1, 2):
            st = const.tile([128, NBT], BF16, name=f"stack{s}")
            nc.sync.dma_start(out=st[:, :], in_=stT_e[ds(128 * s, 128), :])
            stacks[s] = st
        w2l1a = const.tile([128, 256], BF16, name="w2l1a")
        nc.sync.dma_start(out=w2l1a[:, :], in_=w2l1a_e[:, :])
        w2l2a = const.tile([128, 256], BF16, name="w2l2a")
        nc.sync.dma_start(out=w2l2a[:, :], in_=w2l2a_e[:, :])
        cmat = const.tile([128, 385], BF16, name="cmat")
        nc.sync.dma_start(out=cmat[:, :], in_=cmat_e[:, :])
        w1e4 = const.tile([128, 2 * H], BF16, name="w1e4")
        nc.sync.dma_start(out=w1e4[:, :], in_=w1e4_e[:, :])
        for s in (3, 4):
            st = const.tile([128, NBT], BF16, name=f"stack{s}")
            nc.sync.dma_start(out=st[:, :], in_=stT_e[ds(128 * s, 128), :])
            stacks[s] = st
        st5 = const.tile([128, NBT], BF16, name="stack5")
        for r in range(4):
            nc.sync.dma_start(out=st5[ds(32 * r, 32), :], in_=stT_e[ds(640, 32), :])
        stacks[5] = st5
        w2l1e = const.tile([128, 256], BF16, name="w2l1e")
        nc.sync.dma_start(out=w2l1e[:, :], in_=w2l1e_e[:, :])
        w2l2e = const.tile([128, 256], BF16, name="w2l2e")
        nc.sync.dma_start(out=w2l2e[:, :], in_=w2l2e_e[:, :])
        qvT = const.tile([A, NBT], BF16, name="qvT")
        nc.sync.dma_start(out=qvT[:, :], in_=qvT_e[:, :])

        ones10 = cmat[0:A, 128:192]     # [10, 64] all ones
        dotsel = cmat[0:E, 256:321]     # [64, 65]: col 64 = ones
        i64lo = cmat[E:128, 321:385]    # [64@64:128, 64] identity

        b1a_sb = bvec[:, 0:4]
        b1e_sb = bvec[:, 4:8]
        wab_sb = bvec[:, 8:9]
        zb_sb = bvec[0:E, 9:10]
        web_sb = bvec[:, 10:11]
        w2ab_sb = bvec[0:E, 11:12]
        w2eb_sb = bvec[0:E, 12:13]
        ob_sb = bvec[:, 13:14]

        zeros64 = const.tile([E, NF], F32, name="zeros64")
        nc.gpsimd.memset(zeros64[:, :], 0.0)

        relu_ctr = [0]

        def relu_op(dst, src, bias_ap):
            i = relu_ctr[0] % 11
            relu_ctr[0] += 1
            if i in (0, 2, 4, 6, 8):
                nc.scalar.activation(dst, src, AF.Relu, bias=bias_ap)
            else:
                nc.vector.tensor_scalar(dst, src, bias_ap, 0.0, OP.add, OP.max)

        # ---------------- main loop over bt tiles ----------------
        for t in range(NT):
            btsl = ds(NF * t, NF)
            # persistent per-tile accumulators
            b56 = pacc.tile([128, 2 * NF], F32, space="PSUM", name="b56", tag="b56")
            pl2a_w = b56[0:E, 0:NF]          # 20 ally h2 matmuls
            pwa_slot = [b56[E:128, 0:NF], b56[E:128, NF:2 * NF]]
            pq_sl = b56[E:128, 0:NF]         # reused after allies drain
            pwe = pacc.tile([128, NF], F32, space="PSUM", name="pwe", tag="pwe")
            pl2e = pacc.tile([128, NF], F32, space="PSUM", name="pl2e", tag="pl2e")

            ctr = {"b": 0, "we": 0, "l2a": 0, "b2": 0, "l2e": 0}
            N_B = 2 * NAG + A + 1   # b riders + ally folds + he fold
            N_WE = 2 * NE
            N_L2A = 2 * A
            N_B2 = 2 * NAG + 1      # b2a + b2e riders + dot
            N_L2E = 2 * NE

            hs = {}        # (s, mc, r) -> h tile
            qts = {}       # pair -> qv broadcast tile

            def l1_pass(s, mc):
                # stack 5: single pass carries all 4 mc chunks (data
                # replicated across row groups); call only with mc=0.
                riders = range(4) if s == 5 else range(min(4, NAG - 4 * s))
                for r in riders:
                    mcr = r if s == 5 else mc
                    ag = 4 * s + (0 if s == 5 else r)
                    isally = ag < A
                    pht = ph.tile([128, NF], F32, space="PSUM",
                                  name=f"ph_{t}_{s}_{mcr}_{r}", tag="ph")
                    _mm(nc, pht[:, :],
                        (w1a4 if isally else w1e4)[ds(32 * r, 32), ds(128 * mcr, 128)],
                        stacks[s][ds(32 * r, 32), btsl],
                        start=True, stop=True, tile_position=(32 * r, 0))
                    ht = hpool.tile([128, NF], BF16,
                                    name=f"h_{t}_{s}_{mcr}_{r}", tag="h")
                    relu_op(ht[:, :], pht[:, :],
                            (b1a_sb if isally else b1e_sb)[:, ds(mcr, 1)])
                    hs[(s, mcr, 0 if s == 5 else r)] = ht

            def ally_ag(s, r):
                ag = 4 * s + r
                h1 = [hs[(s, 0, r)], hs[(s, 1, r)]]
                h2 = [hs[(s, 2, r)], hs[(s, 3, r)]]
                psl = pwa_slot[ag % 2]
                for kc in range(2):
                    # one pass: per-agent w (cols 64:128, issued first so its
                    # slot-WAR clears before streaming) + b rider (cols 0:64)
                    _mm(nc, psl, w2l1a[:, ds(128 * kc, E)],
                        h1[kc][:, :], start=(kc == 0), stop=(kc == 1),
                        tile_position=(0, E), skip_group_check=True)
                    _mm(nc, pwe[0:E, :], w2l1a[:, ds(128 * kc + E, E)],
                        h1[kc][:, :], start=(ctr["b"] == 0), stop=False,
                        tile_position=(0, 0), skip_group_check=True)
                    ctr["b"] += 1
                for kc in range(2):
                    # one pass: pl2a-w (cols 0:64) + zero-padded M=64 bias
                    # rider (cols 64:128; only col 64 nonzero) — same PE
                    # tile config so they co-stream
                    _mm(nc, pl2a_w, w2l2a[:, ds(128 * kc, E)],
                        h2[kc][:, :], start=(ctr["l2a"] == 0),
                        stop=(ctr["l2a"] == N_L2A - 1),
                        tile_position=(0, 0), skip_group_check=True)
                    ctr["l2a"] += 1
                    _mm(nc, pl2e[ds(E, E), :], w2l2a[:, ds(128 * kc + E, E)],
                        h2[kc][:, :], start=(ctr["b2"] == 0), stop=False,
                        tile_position=(0, E), skip_group_check=True)
                    ctr["b2"] += 1
                if r % 2 == 1:
                    pair = ag // 2
                    qt, abs_t = qts[pair]
                    nc.scalar.activation(abs_t[E:128, :], b56[E:128, :],
                                         AF.Abs, bias=wab_sb[E:128, :])
                    prod = mix.tile([128, 2 * NF], BF16,
                                    name=f"prod_{t}_{pair}", tag="prod")
                    nc.gpsimd.tensor_mul(prod[E:128, :], abs_t[E:128, :],
                                         qt[E:128, :])
                    for half in range(2):
                        _mm(nc, pwe[0:E, :], i64lo,
                            prod[ds(E, E), ds(NF * half, NF)],
                            start=False, stop=False,
                            tile_position=(E, 0), skip_group_check=True)
                        ctr["b"] += 1

            def enemy_ag(s, r):
                ag = 4 * s + (0 if s == 5 else r)
                h1 = [hs[(s, 0, r)], hs[(s, 1, r)]]
                h2 = [hs[(s, 2, r)], hs[(s, 3, r)]]
                for kc in range(2):
                    _mm(nc, pwe[0:E, :], w2l1e[:, ds(128 * kc, E)],
                        h1[kc][:, :], start=(ctr["b"] == 0), stop=False,
                        tile_position=(0, 0), skip_group_check=True)
                    ctr["b"] += 1
                    _mm(nc, pwe[ds(E, E), :], w2l1e[:, ds(128 * kc + E, E)],
                        h1[kc][:, :], start=(ctr["we"] == 0),
                        stop=(ctr["we"] == N_WE - 1),
                        tile_position=(0, E), skip_group_check=True)
                    ctr["we"] += 1
                for kc in range(2):
                    # one pass: pl2e-w (cols 0:64) + padded b2e rider
                    # (cols 64:128) accumulating onto the b2a partials
                    _mm(nc, pl2e[0:E, :], w2l2e[:, ds(128 * kc, E)],
                        h2[kc][:, :], start=(ctr["l2e"] == 0),
                        stop=(ctr["l2e"] == N_L2E - 1),
                        tile_position=(0, 0), skip_group_check=True)
                    ctr["l2e"] += 1
                    _mm(nc, pl2e[ds(E, E), :], w2l2e[:, ds(128 * kc + E, E)],
                        h2[kc][:, :], start=False, stop=False,
                        tile_position=(0, E), skip_group_check=True)

            def prefetch_qt(s):
                # queue qv broadcasts for pairs whose agents sit in stack s
                for pair in range(5):
                    if 2 * pair // 4 == s:
                        qt = qbp.tile([128, 2 * NF], BF16,
                                      name=f"qvb_{t}_{pair}", tag="qvb")
                        nc.sync.dma_start(out=qt[E:128, 0:NF],
                                          in_=qvB_e[2 * pair, :, btsl])
                        nc.sync.dma_start(out=qt[E:128, NF:2 * NF],
                                          in_=qvB_e[2 * pair + 1, :, btsl])
                        abs_t = mix.tile([128, 2 * NF], BF16,
                                         name=f"abs_{t}_{pair}", tag="abs")
                        qts[pair] = (qt, abs_t)

            # -------- software-pipelined issue: L1(s) between L2(s-1) ----
            for si in range(7):
                lws = []
                if si < 6:
                    prefetch_qt(si)
                    lws = [(si, mc) for mc in (range(1) if si == 5 else range(4))]
                ags = []
                if si >= 1:
                    sp = si - 1
                    ags = [(sp, r) for r in
                           (range(1) if sp == 5 else range(min(4, NAG - 4 * sp)))]
                for k in range(max(len(lws), len(ags))):
                    if k < len(lws):
                        l1_pass(*lws[k])
                    if k < len(ags):
                        sp, r = ags[k]
                        if 4 * sp + r < A and sp < 5:
                            ally_ag(sp, r)
                        else:
                            enemy_ag(sp, r)

            # ---------------- mixing ----------------
            _mm(nc, pq_sl, ones10, qvT[:, btsl], start=True, stop=True,
                tile_position=(0, E), skip_group_check=True)
            we_t = mix.tile([128, NF], BF16, name=f"we_{t}", tag="we")
            nc.scalar.activation(we_t[E:128, :], pwe[E:128, :], AF.Abs,
                                 bias=web_sb[E:128, :])
            he_t = mix.tile([128, NF], BF16, name=f"he_{t}", tag="he")
            nc.vector.tensor_mul(he_t[E:128, :], we_t[E:128, :], pq_sl)
            _mm(nc, pwe[0:E, :], i64lo, he_t[ds(E, E), :],
                start=False, stop=True, tile_position=(E, 0),
                skip_group_check=True)
            z = mix.tile([E, NF], F32, name=f"z_{t}", tag="z")
            nc.vector.tensor_scalar(z[:, :], pwe[0:E, :], zb_sb, None, OP.add)
            # elu(z) = max(z, exp(min(z,0)) - 1)
            tmin = mix.tile([E, NF], F32, name=f"tmin_{t}", tag="tmin")
            nc.vector.tensor_scalar_min(tmin[:, :], z[:, :], 0.0)
            texp = mix.tile([E, NF], F32, name=f"texp_{t}", tag="texp")
            nc.scalar.activation(texp[:, :], tmin[:, :], AF.Exp)
            hidden = mix.tile([E, NF], BF16, name=f"hidden_{t}", tag="hidden")
            nc.vector.scalar_tensor_tensor(hidden[:, :], texp[:, :], -1.0,
                                           z[:, :], OP.add, OP.max)
            w2a_t = mix.tile([E, NF], BF16, name=f"w2a_{t}", tag="w2a")
            nc.scalar.activation(w2a_t[:, :], pl2a_w, AF.Abs, bias=w2ab_sb)
            w2e_t = mix.tile([E, NF], BF16, name=f"w2e_{t}", tag="w2e")
            nc.scalar.activation(w2e_t[:, :], pl2e[0:E, :], AF.Abs,
                                 bias=w2eb_sb)
            w2s = mix.tile([E, NF], BF16, name=f"w2s_{t}", tag="w2s")
            nc.gpsimd.tensor_add(w2s[:, :], w2a_t[:, :], w2e_t[:, :])
            prodf = mix.tile([E, NF], BF16, name=f"prodf_{t}", tag="prodf")
            nc.vector.tensor_mul(prodf[:, :], hidden[:, :], w2s[:, :])
            _mm(nc, pl2e[0:E + 1, :], dotsel, prodf[:, :], start=False,
                stop=True, skip_group_check=True)
            o_sb = mix.tile([128, NF], F32, name=f"o_{t}", tag="o")
            nc.scalar.activation(o_sb[E:E + 1, :], pl2e[E:E + 1, :],
                                 AF.Identity, bias=ob_sb[E:E + 1, :])
            nc.sync.dma_start(out=out_e[btsl].unsqueeze(0), in_=o_sb[E:E + 1, :])

    return nc


_BUILT = None


def _get_nc():
    global _BUILT
    if _BUILT is None:
        _BUILT = build()
        _BUILT.finalize()
    return _BUILT


def _prep_in_maps(inputs):
    qv = np.ascontiguousarray(np.asarray(inputs["qvals"], dtype=np.float32)).reshape(BT, A)
    st = np.ascontiguousarray(np.asarray(inputs["states"], dtype=np.float32)).reshape(BT, SD)
    f32 = np.float32
    g = {n: np.asarray(inputs[n], dtype=f32) for n in W_NAMES}
    bias = {n: np.asarray(inputs[n], dtype=f32) for n in
            ["l1a_b1", "l1a_b2", "l1e_b1", "l1e_b2",
             "l2a_b1", "l2a_b2", "l2e_b1", "l2e_b2"]}
    w1a4 = np.tile(np.concatenate([g["l1a_w1"], g["l2a_w1"]], axis=1), (4, 1))
    w1e4 = np.tile(np.concatenate([g["l1e_w1"], g["l2e_w1"]], axis=1), (4, 1))
    w2l1a = np.concatenate([g["l1a_w2"][0:128], g["l1a_w2"][128:256]], axis=1)
    # enemy layer-1 W2 with output cols reordered to [b | w]
    l1e_bw = np.concatenate([g["l1e_w2"][:, E:], g["l1e_w2"][:, :E]], axis=1)
    w2l1e = np.concatenate([l1e_bw[0:128], l1e_bw[128:256]], axis=1)
    def pad_l2(w):  # [256, 65] -> [128, 256] with zero-padded bias cols
        p = np.zeros((256, 128), f32)
        p[:, 0:E] = w[:, :E]
        p[:, E] = w[:, E]
        return np.concatenate([p[0:128], p[128:256]], axis=1)
    w2l2a = pad_l2(g["l2a_w2"])
    w2l2e = pad_l2(g["l2e_w2"])
    bvec = np.zeros((128, 14), f32)
    bvec[:, 0:4] = np.concatenate([bias["l1a_b1"], bias["l2a_b1"]]).reshape(4, 128).T
    bvec[:, 4:8] = np.concatenate([bias["l1e_b1"], bias["l2e_b1"]]).reshape(4, 128).T
    bvec[0:E, 8] = bias["l1a_b2"][:E]
    bvec[E:128, 8] = bias["l1a_b2"][:E]
    bvec[0:E, 9] = A * bias["l1a_b2"][E:] + NE * bias["l1e_b2"][E:]
    bvec[E:128, 10] = NE * bias["l1e_b2"][:E]
    bvec[0:E, 11] = A * bias["l2a_b2"][:E]
    bvec[0:E, 12] = NE * bias["l2e_b2"][:E]
    bvec[E, 13] = A * bias["l2a_b2"][E] + NE * bias["l2e_b2"][E]
    cmat = np.zeros((128, 385), f32)
    cmat[0:A, 128:256] = 1.0                      # ones10
    cmat[0:E, 256 + E] = 1.0                      # dotsel col 64
    cmat[E:128, 321:385] = np.eye(E, dtype=f32)   # i64lo
    import ml_dtypes
    bf16 = ml_dtypes.bfloat16
    wmaps = {
        "w1a4": np.ascontiguousarray(w1a4).astype(bf16),
        "w1e4": np.ascontiguousarray(w1e4).astype(bf16),
        "w2l1a": np.ascontiguousarray(w2l1a).astype(bf16),
        "w2l1e": np.ascontiguousarray(w2l1e).astype(bf16),
        "w2l2a": np.ascontiguousarray(w2l2a).astype(bf16),
        "w2l2e": np.ascontiguousarray(w2l2e).astype(bf16),
        "bvec": bvec, "cmat": cmat.astype(bf16),
    }
    in_maps = []
    for c in range(NCORES):
        sl = slice(c * NBT, (c + 1) * NBT)
        qvc = np.ascontiguousarray(qv[sl].T).astype(bf16)  # [A, NBT]
        m = {
            "qvT": qvc,
            "qvB": np.ascontiguousarray(
                np.broadcast_to(qvc[:, None, :], (A, E, NBT))),
            "statesT": np.ascontiguousarray(st[sl].T).astype(bf16),
        }
        m.update(wmaps)
        in_maps.append(m)
    return in_maps


def run(inputs, **kw):
    nc = _get_nc()
    in_maps = _prep_in_maps(inputs)
    res = run_bass_kernel_spmd(nc, in_maps, list(range(NCORES)), **kw)
    out = np.concatenate([
        np.asarray(res.results[i]["out"], dtype=np.float32).reshape(NBT)
        for i in range(NCORES)])
    return out.reshape(B, T, 1), res


def kernel(**inputs):
    out, _ = run(inputs)
    return out
